# revision 14
# baseline (speedup 1.0000x reference)
"""LPKT knowledge-tracing kernel for 8x Trainium2 NeuronCores.

Data-parallel over batch: B=32 -> 4 batches per core. Per core the recurrent
state h [4, C=256, K=128] lives in SBUF as hT [K=128 partitions, (b,c)=1024
free] in bf16.  All matmuls are bf16 (1 HW pass + 1 cyc/row vs fp32's 2
passes at 4 cyc/row, and ~80ns LDWEIGHTS instead of ~440ns).  All
elementwise work is on DVE: gamma*h uses tensor_tensor (which the compiler
runs in the 2x bf16 mode, ~200ns per [128,256]); h_new and the h_tilde
accumulation need scalar_tensor_tensor (per-batch scalar / accum_out),
which only runs at 1x.  GpSimd is deliberately idle: it shares SBUF ports
with DVE, so offloading bulk elementwise there is net negative.

q rows are broadcast across all 128 partitions by DMA (stride-0 source via
AP.partition_broadcast), one 16-step window (4MB) at a time, double
buffered -- the descriptors fan out over all 16 DMA engines, so no compute
engine spends cycles on replication.

The 4 batches are processed as TWO independent 2-batch streams per step so
stream B's gate chain (PE matmul -> sigmoid -> LG -> W4b matmul -> u)
overlaps stream A's state-update tail.  h_tilde columns accumulate into a
persistent HT_all buffer [K, 4*(T+1)]; gate matmuls read their [K,2] slice
directly and the y head (W5 + sigmoid + reduce) runs once, batched, after
the loop.
"""

import numpy as np

B, S = 32, 128
NUM_Q, NUM_C = 10000, 256
K = 128
C = NUM_C
NCORES = 8
BL = B // NCORES  # 4 batches per core
T = S - 1  # 127 recurrence steps
QW = 16  # q broadcast window, steps
NWIN = S // QW

_cache = {}


def _build():
    import concourse.bass as bass  # noqa: F401
    import concourse.mybir as mybir
    import concourse.tile as tile
    from concourse import bacc

    fp32 = mybir.dt.float32
    bf16 = mybir.dt.bfloat16
    AF = mybir.ActivationFunctionType
    OP = mybir.AluOpType

    nc = bacc.Bacc()

    # ---------------- DRAM I/O ----------------
    d = {}

    def din(name, shape, dt_=bf16):
        t = nc.dram_tensor(name, shape, dt_, kind="ExternalInput")
        d[name] = t
        return t

    din("eT", [K, S * BL])       # e_emb^T, free layout (s, b) s-major
    din("atT", [K, S * BL])
    din("itT", [K, S * BL])
    din("a_row", [1, S * BL])
    din("h0T4", [K, BL * C])
    q_dram = nc.dram_tensor("qD", [S, BL * C], bf16, kind="ExternalInput")
    for w in ["W1a", "W1b", "W2a2", "W2b2", "W2c2", "W2d2",
              "W3a", "W3b", "W3c", "W3d", "W4a", "W4b", "W4c",
              "W5a", "W5b"]:
        din(w, [K, K])
    for w in ["w1c", "b1r", "b2r2", "b3r", "b4r", "b5r"]:
        din(w, [1, K])
    din("ones512", [1, 512])
    din("I128", [K, K])
    din("ones128c", [K, 1])
    y_dram = nc.dram_tensor("y_out", [1, BL * T], fp32, kind="ExternalOutput")

    from contextlib import ExitStack

    with tile.TileContext(nc) as tc, ExitStack() as ctx:
        singles = ctx.enter_context(tc.tile_pool(name="singles", bufs=1))
        state = ctx.enter_context(tc.tile_pool(name="state", bufs=1))
        sm = ctx.enter_context(tc.tile_pool(name="sm", bufs=3))
        qw = ctx.enter_context(tc.tile_pool(name="qw", bufs=2))
        pp = ctx.enter_context(tc.tile_pool(name="pp", bufs=2, space="PSUM"))
        psm = ctx.enter_context(tc.tile_pool(name="psm", bufs=2, space="PSUM"))

        # ---------------- load everything to SBUF ----------------
        sb = {}
        for name, dt_ in d.items():
            if name == "h0T4":
                continue  # loaded straight into the state tile below
            t_ = singles.tile(list(dt_.shape), dt_.dtype, tag=name)
            nc.sync.dma_start(out=t_[:], in_=dt_[:])
            sb[name] = t_

        # recurrent state h, DMA'd straight from the prepped h0 tile
        s_h = state.tile([K, BL * C], bf16, tag="h")
        nc.sync.dma_start(out=s_h[:], in_=d["h0T4"][:])

        # q windows: all 128 partitions get a copy of q rows [16w, 16w+16)
        qwin = [None] * NWIN

        def qwin_load(w):
            wt = qw.tile([K, QW * BL * C], bf16, tag="qwin")
            src = q_dram[w * QW:(w + 1) * QW, :].partition_broadcast(K)
            nc.sync.dma_start(out=wt[:], in_=src)
            qwin[w] = wt

        qwin_load(0)
        qwin_load(1)

        def qsl(t):
            # [K, 1024] replicated q row for step t
            base = (t % QW) * BL * C
            return qwin[t // QW][:, base:base + BL * C]

        # collapse the ~30 input-DMA dependencies
        tc.strict_bb_all_engine_barrier()

        s_gam = state.tile([K, BL * C], bf16, tag="gam")
        s_m = state.tile([K, BL * C], bf16, tag="m")
        # h_tilde history: block t (cols 4t:4t+4) = h_tilde at step t
        s_HT = state.tile([K, (T + 1) * BL], bf16, tag="HT")

        # ---------------- precompute: allT, Z23, U4, Y5 ----------------
        p_all = pp.tile([K, 512], fp32, tag="pbig", bufs=1)
        nc.tensor.matmul(out=p_all[:], lhsT=sb["W1a"][:], rhs=sb["eT"][:],
                         start=True, stop=False)
        nc.tensor.matmul(out=p_all[:], lhsT=sb["W1b"][:], rhs=sb["atT"][:],
                         start=False, stop=False)
        nc.tensor.matmul(out=p_all[:], lhsT=sb["w1c"][:], rhs=sb["a_row"][:],
                         start=False, stop=False)
        nc.tensor.matmul(out=p_all[:], lhsT=sb["b1r"][:],
                         rhs=sb["ones512"][:], start=False, stop=True)
        s_allT = singles.tile([K, 512], bf16, tag="allT")
        nc.vector.tensor_copy(out=s_allT[:], in_=p_all[:])

        # Z23[k, t, g, b2, b]: gate g in {2,3}, stream b2, batch-in-stream b
        s_Z23 = singles.tile([K, T, 2, 2, 2], bf16, tag="Z23")

        def precompute_z(Wpre, Wit, Wlearn, brow, g):
            ptile = pp.tile([K, T * BL], fp32, tag="pbig", bufs=1)
            nc.tensor.matmul(out=ptile[:], lhsT=sb[Wit][:],
                             rhs=sb["itT"][:, 0:T * BL], start=True, stop=False)
            nc.tensor.matmul(out=ptile[:, BL:T * BL], lhsT=sb[Wpre][:],
                             rhs=s_allT[:, 0:(T - 1) * BL],
                             start=False, stop=False, skip_group_check=True)
            nc.tensor.matmul(out=ptile[:], lhsT=sb[Wlearn][:],
                             rhs=s_allT[:, 0:T * BL], start=False, stop=False)
            nc.tensor.matmul(out=ptile[:], lhsT=sb[brow][:],
                             rhs=sb["ones512"][:, 0:T * BL], start=False,
                             stop=True)
            nc.vector.tensor_copy(
                out=s_Z23[:, :, g, :, :],
                in_=ptile[:].rearrange("k (t b2 b) -> k t b2 b", b2=2, b=2))

        precompute_z("W2a2", "W2b2", "W2c2", "b2r2", 0)
        precompute_z("W3a", "W3b", "W3c", "b3r", 1)

        # U4[k, (t,b)] = it@W4c + b4
        p_u4 = pp.tile([K, T * BL], fp32, tag="pbig", bufs=1)
        nc.tensor.matmul(out=p_u4[:], lhsT=sb["W4c"][:],
                         rhs=sb["itT"][:, 0:T * BL], start=True, stop=False)
        nc.tensor.matmul(out=p_u4[:], lhsT=sb["b4r"][:],
                         rhs=sb["ones512"][:, 0:T * BL], start=False, stop=True)
        s_U4 = singles.tile([K, T * BL], bf16, tag="U4")
        nc.vector.tensor_copy(out=s_U4[:], in_=p_u4[:])

        # Y5[k, (t,b)] = e_emb[t+1]@W5a + b5
        p_y5 = pp.tile([K, T * BL], fp32, tag="pbig", bufs=1)
        nc.tensor.matmul(out=p_y5[:], lhsT=sb["W5a"][:],
                         rhs=sb["eT"][:, BL:S * BL], start=True, stop=False)
        nc.tensor.matmul(out=p_y5[:], lhsT=sb["b5r"][:],
                         rhs=sb["ones512"][:, 0:T * BL], start=False, stop=True)
        s_Y5 = singles.tile([K, T * BL], fp32, tag="Y5")
        nc.vector.tensor_copy(out=s_Y5[:], in_=p_y5[:])

        # ---------------- h_tilde init (with q_0) ----------------
        for b in range(BL):
            cs = slice(b * C, (b + 1) * C)
            nc.vector.scalar_tensor_tensor(
                out=s_m[:, cs], in0=s_h[:, cs], scalar=0.0,
                in1=qsl(0)[:, cs], op0=OP.bypass, op1=OP.mult,
                accum_out=s_HT[:, b:b + 1])

        # ---------------- the recurrence (two 2-batch streams) ----------
        for t in range(T):
            if t % QW == 0 and t > 0 and (t // QW + 1) < NWIN:
                qwin_load(t // QW + 1)

            ps = psm.tile([K, 16], fp32, tag="small")
            # ---- gate chains for BOTH streams first (small PE matmuls) ----
            sts = []
            for s2 in range(2):
                o = s2 * 8
                ht_sl = s_HT[:, t * BL + 2 * s2:t * BL + 2 * s2 + 2]
                nc.tensor.matmul(out=ps[:, o:o + 2], lhsT=sb["W2d2"][:],
                                 rhs=ht_sl, start=True, stop=False)
                nc.tensor.matmul(out=ps[:, o + 2:o + 4], lhsT=sb["W3d"][:],
                                 rhs=ht_sl, start=True, stop=False)
                # += Z23[t] on PE (identity pass-through)
                nc.tensor.matmul(out=ps[:, o:o + 4], lhsT=sb["I128"][:],
                                 rhs=s_Z23[:, t, :, s2, :], start=False,
                                 stop=True, skip_group_check=True)
            # big gamma_f preact matmuls AFTER all gate matmuls so neither
            # stream's gate chain waits behind a 512-row matmul
            pP = []
            for s2 in range(2):
                pPs = pp.tile([K, 512], fp32, tag=f"pP{s2}")
                nc.tensor.matmul(out=pPs[:], lhsT=sb["W4a"][:],
                                 rhs=s_h[:, s2 * 512:(s2 + 1) * 512],
                                 start=True, stop=True)
                pP.append(pPs)
            for s2 in range(2):
                o = s2 * 8
                s23 = sm.tile([K, 4], bf16, tag=f"s23{s2}")
                nc.scalar.activation(out=s23[:], in_=ps[:, o:o + 4],
                                     func=AF.Sigmoid)
                # LG on the otherwise-idle GpSimd queue
                LGT = sm.tile([K, 2], bf16, tag=f"LGT{s2}")
                nc.gpsimd.tensor_mul(out=LGT[:], in0=s23[:, 0:2],
                                     in1=s23[:, 2:4])
                # u = LG @ W4b + U4[t] (U4 added on PE)
                nc.tensor.matmul(out=ps[:, o + 4:o + 6], lhsT=sb["W4b"][:],
                                 rhs=LGT[:], start=True, stop=False)
                nc.tensor.matmul(
                    out=ps[:, o + 4:o + 6], lhsT=sb["I128"][:],
                    rhs=s_U4[:, t * BL + 2 * s2:t * BL + 2 * s2 + 2],
                    start=False, stop=True)
                uT = sm.tile([K, 2], fp32, tag=f"uT{s2}")
                nc.scalar.copy(out=uT[:], in_=ps[:, o + 4:o + 6])
                sts.append((LGT, uT))

            # ---- state-update tails, interleaved batch-wise across streams
            for b in range(2):
                for s2 in range(2):
                    LGT, uT = sts[s2]
                    gb = 2 * s2 + b
                    cs = slice(gb * C, (gb + 1) * C)
                    nc.scalar.activation(out=s_gam[:, cs],
                                         in_=pP[s2][:, b * C:(b + 1) * C],
                                         func=AF.Sigmoid, bias=uT[:, b:b + 1])
                for s2 in range(2):
                    LGT, uT = sts[s2]
                    gb = 2 * s2 + b
                    cs = slice(gb * C, (gb + 1) * C)
                    # m = gamma * h
                    nc.vector.tensor_mul(out=s_m[:, cs], in0=s_gam[:, cs],
                                         in1=s_h[:, cs])
                    # h_new = q_e * LG + m
                    nc.vector.scalar_tensor_tensor(
                        out=s_h[:, cs], in0=qsl(t)[:, cs],
                        scalar=LGT[:, b:b + 1], in1=s_m[:, cs],
                        op0=OP.mult, op1=OP.add)
                    # h_tilde accumulation with q_{t+1}
                    col = (t + 1) * BL + gb
                    nc.vector.scalar_tensor_tensor(
                        out=s_m[:, cs], in0=s_h[:, cs], scalar=0.0,
                        in1=qsl(t + 1)[:, cs], op0=OP.bypass, op1=OP.mult,
                        accum_out=s_HT[:, col:col + 1])

        # ---------------- y head, batched over all steps ----------------
        p_y = pp.tile([K, T * BL], fp32, tag="pbig", bufs=1)
        nc.tensor.matmul(out=p_y[:], lhsT=sb["W5b"][:],
                         rhs=s_HT[:, BL:(T + 1) * BL], start=True, stop=True)
        tY = singles.tile([K, T * BL], fp32, tag="tY")
        nc.vector.tensor_add(out=tY[:], in0=p_y[:], in1=s_Y5[:])
        sY = singles.tile([K, T * BL], bf16, tag="sY")
        nc.scalar.activation(out=sY[:], in_=tY[:], func=AF.Sigmoid)
        p_ys = psm.tile([1, T * BL], fp32, tag="yacc", bufs=1)
        nc.tensor.matmul(out=p_ys[:], lhsT=sb["ones128c"][:], rhs=sY[:],
                         start=True, stop=True)
        s_y = singles.tile([1, T * BL], fp32, tag="yout")
        nc.vector.tensor_copy(out=s_y[:], in_=p_ys[:])
        nc.sync.dma_start(out=y_dram[:], in_=s_y[:])

    nc.compile()
    return nc


def _prep_inputs(inputs):
    """Host-side sharding + layout prep. Returns per-core input dicts."""
    import ml_dtypes

    bf = ml_dtypes.bfloat16
    f32 = np.float32
    e_idx = np.asarray(inputs["e_data"]).astype(np.int64)
    at_idx = np.asarray(inputs["at_data"]).astype(np.int64)
    it_idx = np.asarray(inputs["it_data"]).astype(np.int64)
    a_data = np.asarray(inputs["a_data"], dtype=f32)
    q_matrix = np.asarray(inputs["q_matrix"], dtype=f32)
    e_E = np.asarray(inputs["e_E"], dtype=bf)
    at_E = np.asarray(inputs["at_E"], dtype=bf)
    it_E = np.asarray(inputs["it_E"], dtype=bf)
    W1 = np.asarray(inputs["W1"], dtype=f32)
    W2 = np.asarray(inputs["W2"], dtype=f32)
    W3 = np.asarray(inputs["W3"], dtype=f32)
    W4 = np.asarray(inputs["W4"], dtype=f32)
    W5 = np.asarray(inputs["W5"], dtype=f32)
    h0 = np.asarray(inputs["h0"], dtype=f32)

    def bfc(x):
        return np.ascontiguousarray(np.asarray(x, dtype=bf))

    shared = {
        "W1a": bfc(W1[0:K]), "W1b": bfc(W1[K:2 * K]),
        "w1c": bfc(W1[2 * K:].sum(0)[None, :]),
        "b1r": bfc(np.asarray(inputs["b1"], dtype=f32)[None, :]),
        "W2a2": bfc(2 * W2[0:K]), "W2b2": bfc(2 * W2[K:2 * K]),
        "W2c2": bfc(2 * W2[2 * K:3 * K]), "W2d2": bfc(2 * W2[3 * K:]),
        "b2r2": bfc(2 * np.asarray(inputs["b2"], dtype=f32)[None, :]),
        "W3a": bfc(W3[0:K]), "W3b": bfc(W3[K:2 * K]),
        "W3c": bfc(W3[2 * K:3 * K]), "W3d": bfc(W3[3 * K:]),
        "b3r": bfc(np.asarray(inputs["b3"], dtype=f32)[None, :]),
        "W4a": bfc(W4[0:K]), "W4b": bfc(W4[K:2 * K]), "W4c": bfc(W4[2 * K:]),
        "b4r": bfc(np.asarray(inputs["b4"], dtype=f32)[None, :]),
        "W5a": bfc(W5[0:K]), "W5b": bfc(W5[K:]),
        "b5r": bfc(np.asarray(inputs["b5"], dtype=f32)[None, :]),
        "ones512": bfc(np.ones((1, 512), f32)),
        "ones128c": bfc(np.ones((K, 1), f32)),
        "I128": bfc(np.eye(K, dtype=f32)),
        "h0T4": bfc(np.tile(np.ascontiguousarray(h0.T), (1, BL))),
    }

    in_maps = []
    for g in range(NCORES):
        bg = slice(g * BL, (g + 1) * BL)
        e_emb = e_E[e_idx[bg]]          # [4, S, K] bf16
        at_emb = at_E[at_idx[bg]]
        it_emb = it_E[it_idx[bg]]
        q_all = q_matrix[e_idx[bg]]     # [4, S, C] f32
        m = dict(shared)
        # [K, (s, b)] s-major layouts
        m["eT"] = bfc(e_emb.transpose(2, 1, 0).reshape(K, S * BL))
        m["atT"] = bfc(at_emb.transpose(2, 1, 0).reshape(K, S * BL))
        m["itT"] = bfc(it_emb.transpose(2, 1, 0).reshape(K, S * BL))
        m["qD"] = bfc(q_all.transpose(1, 0, 2).reshape(S, BL * C))
        m["a_row"] = bfc(a_data[bg].T.reshape(1, S * BL))
        in_maps.append(m)
    return in_maps


def _run(inputs, trace=False):
    from concourse.bass_utils import run_bass_kernel_spmd

    if "nc" not in _cache:
        _cache["nc"] = _build()
    nc = _cache["nc"]
    in_maps = _prep_inputs(inputs)
    res = run_bass_kernel_spmd(nc, in_maps, core_ids=list(range(NCORES)),
                               trace=trace)
    pred = np.zeros((B, S), np.float32)
    for g in range(NCORES):
        y = res.results[g]["y_out"].reshape(T, BL)  # [t, b]
        pred[g * BL:(g + 1) * BL, 1:] = y.T / K
    return pred, res


def kernel(**inputs):
    return _run(inputs)[0]


# revision 15
# speedup vs baseline: 1.0660x; 1.0660x over previous
"""LPKT knowledge-tracing kernel for 8x Trainium2 NeuronCores.

Data-parallel over batch: B=32 -> 4 batches per core. Per core the recurrent
state h [4, C=256, K=128] lives in SBUF as hT [K=128 partitions, (b,c)=1024
free] in bf16.  All matmuls are bf16 (1 HW pass + 1 cyc/row vs fp32's 2
passes at 4 cyc/row, and ~80ns LDWEIGHTS instead of ~440ns).  All
elementwise work is on DVE: gamma*h uses tensor_tensor (which the compiler
runs in the 2x bf16 mode, ~200ns per [128,256]); h_new and the h_tilde
accumulation need scalar_tensor_tensor (per-batch scalar / accum_out),
which only runs at 1x.  GpSimd is deliberately idle: it shares SBUF ports
with DVE, so offloading bulk elementwise there is net negative.

q rows are broadcast across all 128 partitions by DMA (stride-0 source via
AP.partition_broadcast), one 16-step window (4MB) at a time, double
buffered -- the descriptors fan out over all 16 DMA engines, so no compute
engine spends cycles on replication.

The 4 batches are processed as TWO independent 2-batch streams per step so
stream B's gate chain (PE matmul -> sigmoid -> LG -> W4b matmul -> u)
overlaps stream A's state-update tail.  h_tilde columns accumulate into a
persistent HT_all buffer [K, 4*(T+1)]; gate matmuls read their [K,2] slice
directly and the y head (W5 + sigmoid + reduce) runs once, batched, after
the loop.
"""

import numpy as np

B, S = 32, 128
NUM_Q, NUM_C = 10000, 256
K = 128
C = NUM_C
NCORES = 8
BL = B // NCORES  # 4 batches per core
T = S - 1  # 127 recurrence steps
QW = 16  # q broadcast window, steps
NWIN = S // QW

_cache = {}


def _build():
    import concourse.bass as bass  # noqa: F401
    import concourse.mybir as mybir
    import concourse.tile as tile
    from concourse import bacc

    fp32 = mybir.dt.float32
    bf16 = mybir.dt.bfloat16
    AF = mybir.ActivationFunctionType
    OP = mybir.AluOpType

    nc = bacc.Bacc()

    # ---------------- DRAM I/O ----------------
    d = {}

    def din(name, shape, dt_=bf16):
        t = nc.dram_tensor(name, shape, dt_, kind="ExternalInput")
        d[name] = t
        return t

    din("eT", [K, S * BL])       # e_emb^T, free layout (s, b) s-major
    din("atT", [K, S * BL])
    din("itT", [K, S * BL])
    din("a_row", [1, S * BL])
    din("h0T4", [K, BL * C])
    q_dram = nc.dram_tensor("qD", [S, BL * C], bf16, kind="ExternalInput")
    for w in ["W1a", "W1b", "W2a2", "W2b2", "W2c2", "W2d2",
              "W3a", "W3b", "W3c", "W3d", "W4a", "W4b", "W4c",
              "W5a", "W5b"]:
        din(w, [K, K])
    for w in ["w1c", "b1r", "b2r2", "b3r", "b4r", "b5r"]:
        din(w, [1, K])
    din("ones512", [1, 512])
    din("I128", [K, K])
    din("ones128c", [K, 1])
    y_dram = nc.dram_tensor("y_out", [1, BL * T], fp32, kind="ExternalOutput")

    from contextlib import ExitStack

    with tile.TileContext(nc) as tc, ExitStack() as ctx:
        singles = ctx.enter_context(tc.tile_pool(name="singles", bufs=1))
        state = ctx.enter_context(tc.tile_pool(name="state", bufs=1))
        sm = ctx.enter_context(tc.tile_pool(name="sm", bufs=3))
        qw = ctx.enter_context(tc.tile_pool(name="qw", bufs=2))
        pp = ctx.enter_context(tc.tile_pool(name="pp", bufs=2, space="PSUM"))
        psm = ctx.enter_context(tc.tile_pool(name="psm", bufs=2, space="PSUM"))

        # ---------------- load everything to SBUF ----------------
        sb = {}
        for name, dt_ in d.items():
            if name == "h0T4":
                continue  # loaded straight into the state tile below
            t_ = singles.tile(list(dt_.shape), dt_.dtype, tag=name)
            nc.sync.dma_start(out=t_[:], in_=dt_[:])
            sb[name] = t_

        # recurrent state h, DMA'd straight from the prepped h0 tile
        s_h = state.tile([K, BL * C], bf16, tag="h")
        nc.sync.dma_start(out=s_h[:], in_=d["h0T4"][:])

        # q windows: all 128 partitions get a copy of q rows [16w, 16w+16)
        qwin = [None] * NWIN

        def qwin_load(w):
            wt = qw.tile([K, QW * BL * C], bf16, tag="qwin")
            src = q_dram[w * QW:(w + 1) * QW, :].partition_broadcast(K)
            nc.sync.dma_start(out=wt[:], in_=src)
            qwin[w] = wt

        qwin_load(0)
        qwin_load(1)

        def qsl(t):
            # [K, 1024] replicated q row for step t
            base = (t % QW) * BL * C
            return qwin[t // QW][:, base:base + BL * C]

        # collapse the ~30 input-DMA dependencies
        tc.strict_bb_all_engine_barrier()

        s_gam = state.tile([K, BL * C], bf16, tag="gam")
        s_m = state.tile([K, BL * C], bf16, tag="m")
        # h_tilde history: block t (cols 4t:4t+4) = h_tilde at step t
        s_HT = state.tile([K, (T + 1) * BL], bf16, tag="HT")

        # ---------------- precompute: allT, Z23, U4, Y5 ----------------
        p_all = pp.tile([K, 512], fp32, tag="pbig", bufs=1)
        nc.tensor.matmul(out=p_all[:], lhsT=sb["W1a"][:], rhs=sb["eT"][:],
                         start=True, stop=False)
        nc.tensor.matmul(out=p_all[:], lhsT=sb["W1b"][:], rhs=sb["atT"][:],
                         start=False, stop=False)
        nc.tensor.matmul(out=p_all[:], lhsT=sb["w1c"][:], rhs=sb["a_row"][:],
                         start=False, stop=False)
        nc.tensor.matmul(out=p_all[:], lhsT=sb["b1r"][:],
                         rhs=sb["ones512"][:], start=False, stop=True)
        s_allT = singles.tile([K, 512], bf16, tag="allT")
        nc.vector.tensor_copy(out=s_allT[:], in_=p_all[:])

        # Z23[k, t, g, b2, b]: gate g in {2,3}, stream b2, batch-in-stream b
        s_Z23 = singles.tile([K, T, 2, 2, 2], bf16, tag="Z23")

        def precompute_z(Wpre, Wit, Wlearn, brow, g):
            ptile = pp.tile([K, T * BL], fp32, tag="pbig", bufs=1)
            nc.tensor.matmul(out=ptile[:], lhsT=sb[Wit][:],
                             rhs=sb["itT"][:, 0:T * BL], start=True, stop=False)
            nc.tensor.matmul(out=ptile[:, BL:T * BL], lhsT=sb[Wpre][:],
                             rhs=s_allT[:, 0:(T - 1) * BL],
                             start=False, stop=False, skip_group_check=True)
            nc.tensor.matmul(out=ptile[:], lhsT=sb[Wlearn][:],
                             rhs=s_allT[:, 0:T * BL], start=False, stop=False)
            nc.tensor.matmul(out=ptile[:], lhsT=sb[brow][:],
                             rhs=sb["ones512"][:, 0:T * BL], start=False,
                             stop=True)
            nc.vector.tensor_copy(
                out=s_Z23[:, :, g, :, :],
                in_=ptile[:].rearrange("k (t b2 b) -> k t b2 b", b2=2, b=2))

        precompute_z("W2a2", "W2b2", "W2c2", "b2r2", 0)
        precompute_z("W3a", "W3b", "W3c", "b3r", 1)

        # U4[k, (t,b)] = it@W4c + b4
        p_u4 = pp.tile([K, T * BL], fp32, tag="pbig", bufs=1)
        nc.tensor.matmul(out=p_u4[:], lhsT=sb["W4c"][:],
                         rhs=sb["itT"][:, 0:T * BL], start=True, stop=False)
        nc.tensor.matmul(out=p_u4[:], lhsT=sb["b4r"][:],
                         rhs=sb["ones512"][:, 0:T * BL], start=False, stop=True)
        s_U4 = singles.tile([K, T * BL], bf16, tag="U4")
        nc.vector.tensor_copy(out=s_U4[:], in_=p_u4[:])

        # Y5[k, (t,b)] = e_emb[t+1]@W5a + b5
        p_y5 = pp.tile([K, T * BL], fp32, tag="pbig", bufs=1)
        nc.tensor.matmul(out=p_y5[:], lhsT=sb["W5a"][:],
                         rhs=sb["eT"][:, BL:S * BL], start=True, stop=False)
        nc.tensor.matmul(out=p_y5[:], lhsT=sb["b5r"][:],
                         rhs=sb["ones512"][:, 0:T * BL], start=False, stop=True)
        s_Y5 = singles.tile([K, T * BL], fp32, tag="Y5")
        nc.vector.tensor_copy(out=s_Y5[:], in_=p_y5[:])

        # ---------------- h_tilde init (with q_0) ----------------
        for b in range(BL):
            cs = slice(b * C, (b + 1) * C)
            nc.vector.scalar_tensor_tensor(
                out=s_m[:, cs], in0=s_h[:, cs], scalar=0.0,
                in1=qsl(0)[:, cs], op0=OP.bypass, op1=OP.mult,
                accum_out=s_HT[:, b:b + 1])

        # ---------------- the recurrence (two 2-batch streams) ----------
        for t in range(T):
            if t % QW == 0 and t > 0 and (t // QW + 1) < NWIN:
                qwin_load(t // QW + 1)

            ps = psm.tile([K, 16], fp32, tag="small")
            # small gate matmuls for BOTH streams first, then the big
            # gamma_f preacts, so neither gate chain waits behind a
            # 512-row matmul in the PE queue
            for s2 in range(2):
                o = s2 * 8
                ht_sl = s_HT[:, t * BL + 2 * s2:t * BL + 2 * s2 + 2]
                nc.tensor.matmul(out=ps[:, o:o + 2], lhsT=sb["W2d2"][:],
                                 rhs=ht_sl, start=True, stop=False)
                nc.tensor.matmul(out=ps[:, o + 2:o + 4], lhsT=sb["W3d"][:],
                                 rhs=ht_sl, start=True, stop=False)
                nc.tensor.matmul(out=ps[:, o:o + 4], lhsT=sb["I128"][:],
                                 rhs=s_Z23[:, t, :, s2, :], start=False,
                                 stop=True, skip_group_check=True)
            pP = []
            for s2 in range(2):
                pPs = pp.tile([K, 512], fp32, tag=f"pP{s2}")
                nc.tensor.matmul(out=pPs[:], lhsT=sb["W4a"][:],
                                 rhs=s_h[:, s2 * 512:(s2 + 1) * 512],
                                 start=True, stop=True)
                pP.append(pPs)
            for s2 in range(2):
                o = s2 * 8
                s23 = sm.tile([K, 4], bf16, tag=f"s23{s2}")
                nc.scalar.activation(out=s23[:], in_=ps[:, o:o + 4],
                                     func=AF.Sigmoid)
                # LG on the otherwise-idle GpSimd queue: never waits behind
                # DVE bulk work
                LGT = sm.tile([K, 2], bf16, tag=f"LGT{s2}")
                nc.gpsimd.tensor_mul(out=LGT[:], in0=s23[:, 0:2],
                                     in1=s23[:, 2:4])

                # u = LG @ W4b + U4[t] (U4 added on PE); psum -> SBUF move on
                # ACT, same queue as the gamma sigmoids that consume it
                nc.tensor.matmul(out=ps[:, o + 4:o + 6], lhsT=sb["W4b"][:],
                                 rhs=LGT[:], start=True, stop=False)
                nc.tensor.matmul(
                    out=ps[:, o + 4:o + 6], lhsT=sb["I128"][:],
                    rhs=s_U4[:, t * BL + 2 * s2:t * BL + 2 * s2 + 2],
                    start=False, stop=True)
                uT = sm.tile([K, 2], fp32, tag=f"uT{s2}")
                nc.scalar.copy(out=uT[:], in_=ps[:, o + 4:o + 6])

                for b in range(2):
                    gb = 2 * s2 + b
                    cs = slice(gb * C, (gb + 1) * C)
                    nc.scalar.activation(out=s_gam[:, cs],
                                         in_=pP[s2][:, b * C:(b + 1) * C],
                                         func=AF.Sigmoid, bias=uT[:, b:b + 1])
                    # m = gamma * h
                    nc.vector.tensor_mul(out=s_m[:, cs], in0=s_gam[:, cs],
                                         in1=s_h[:, cs])
                    # h_new = q_e * LG + m
                    nc.vector.scalar_tensor_tensor(
                        out=s_h[:, cs], in0=qsl(t)[:, cs],
                        scalar=LGT[:, b:b + 1], in1=s_m[:, cs],
                        op0=OP.mult, op1=OP.add)
                    # h_tilde accumulation with q_{t+1}
                    col = (t + 1) * BL + gb
                    nc.vector.scalar_tensor_tensor(
                        out=s_m[:, cs], in0=s_h[:, cs], scalar=0.0,
                        in1=qsl(t + 1)[:, cs], op0=OP.bypass, op1=OP.mult,
                        accum_out=s_HT[:, col:col + 1])

        # ---------------- y head, batched over all steps ----------------
        p_y = pp.tile([K, T * BL], fp32, tag="pbig", bufs=1)
        nc.tensor.matmul(out=p_y[:], lhsT=sb["W5b"][:],
                         rhs=s_HT[:, BL:(T + 1) * BL], start=True, stop=True)
        tY = singles.tile([K, T * BL], fp32, tag="tY")
        nc.vector.tensor_add(out=tY[:], in0=p_y[:], in1=s_Y5[:])
        sY = singles.tile([K, T * BL], bf16, tag="sY")
        nc.scalar.activation(out=sY[:], in_=tY[:], func=AF.Sigmoid)
        p_ys = psm.tile([1, T * BL], fp32, tag="yacc", bufs=1)
        nc.tensor.matmul(out=p_ys[:], lhsT=sb["ones128c"][:], rhs=sY[:],
                         start=True, stop=True)
        s_y = singles.tile([1, T * BL], fp32, tag="yout")
        nc.vector.tensor_copy(out=s_y[:], in_=p_ys[:])
        nc.sync.dma_start(out=y_dram[:], in_=s_y[:])

    nc.compile()
    return nc


def _prep_inputs(inputs):
    """Host-side sharding + layout prep. Returns per-core input dicts."""
    import ml_dtypes

    bf = ml_dtypes.bfloat16
    f32 = np.float32
    e_idx = np.asarray(inputs["e_data"]).astype(np.int64)
    at_idx = np.asarray(inputs["at_data"]).astype(np.int64)
    it_idx = np.asarray(inputs["it_data"]).astype(np.int64)
    a_data = np.asarray(inputs["a_data"], dtype=f32)
    q_matrix = np.asarray(inputs["q_matrix"], dtype=f32)
    e_E = np.asarray(inputs["e_E"], dtype=bf)
    at_E = np.asarray(inputs["at_E"], dtype=bf)
    it_E = np.asarray(inputs["it_E"], dtype=bf)
    W1 = np.asarray(inputs["W1"], dtype=f32)
    W2 = np.asarray(inputs["W2"], dtype=f32)
    W3 = np.asarray(inputs["W3"], dtype=f32)
    W4 = np.asarray(inputs["W4"], dtype=f32)
    W5 = np.asarray(inputs["W5"], dtype=f32)
    h0 = np.asarray(inputs["h0"], dtype=f32)

    def bfc(x):
        return np.ascontiguousarray(np.asarray(x, dtype=bf))

    shared = {
        "W1a": bfc(W1[0:K]), "W1b": bfc(W1[K:2 * K]),
        "w1c": bfc(W1[2 * K:].sum(0)[None, :]),
        "b1r": bfc(np.asarray(inputs["b1"], dtype=f32)[None, :]),
        "W2a2": bfc(2 * W2[0:K]), "W2b2": bfc(2 * W2[K:2 * K]),
        "W2c2": bfc(2 * W2[2 * K:3 * K]), "W2d2": bfc(2 * W2[3 * K:]),
        "b2r2": bfc(2 * np.asarray(inputs["b2"], dtype=f32)[None, :]),
        "W3a": bfc(W3[0:K]), "W3b": bfc(W3[K:2 * K]),
        "W3c": bfc(W3[2 * K:3 * K]), "W3d": bfc(W3[3 * K:]),
        "b3r": bfc(np.asarray(inputs["b3"], dtype=f32)[None, :]),
        "W4a": bfc(W4[0:K]), "W4b": bfc(W4[K:2 * K]), "W4c": bfc(W4[2 * K:]),
        "b4r": bfc(np.asarray(inputs["b4"], dtype=f32)[None, :]),
        "W5a": bfc(W5[0:K]), "W5b": bfc(W5[K:]),
        "b5r": bfc(np.asarray(inputs["b5"], dtype=f32)[None, :]),
        "ones512": bfc(np.ones((1, 512), f32)),
        "ones128c": bfc(np.ones((K, 1), f32)),
        "I128": bfc(np.eye(K, dtype=f32)),
        "h0T4": bfc(np.tile(np.ascontiguousarray(h0.T), (1, BL))),
    }

    in_maps = []
    for g in range(NCORES):
        bg = slice(g * BL, (g + 1) * BL)
        e_emb = e_E[e_idx[bg]]          # [4, S, K] bf16
        at_emb = at_E[at_idx[bg]]
        it_emb = it_E[it_idx[bg]]
        q_all = q_matrix[e_idx[bg]]     # [4, S, C] f32
        m = dict(shared)
        # [K, (s, b)] s-major layouts
        m["eT"] = bfc(e_emb.transpose(2, 1, 0).reshape(K, S * BL))
        m["atT"] = bfc(at_emb.transpose(2, 1, 0).reshape(K, S * BL))
        m["itT"] = bfc(it_emb.transpose(2, 1, 0).reshape(K, S * BL))
        m["qD"] = bfc(q_all.transpose(1, 0, 2).reshape(S, BL * C))
        m["a_row"] = bfc(a_data[bg].T.reshape(1, S * BL))
        in_maps.append(m)
    return in_maps


def _run(inputs, trace=False):
    from concourse.bass_utils import run_bass_kernel_spmd

    if "nc" not in _cache:
        _cache["nc"] = _build()
    nc = _cache["nc"]
    in_maps = _prep_inputs(inputs)
    res = run_bass_kernel_spmd(nc, in_maps, core_ids=list(range(NCORES)),
                               trace=trace)
    pred = np.zeros((B, S), np.float32)
    for g in range(NCORES):
        y = res.results[g]["y_out"].reshape(T, BL)  # [t, b]
        pred[g * BL:(g + 1) * BL, 1:] = y.T / K
    return pred, res


def kernel(**inputs):
    return _run(inputs)[0]


# revision 16
# speedup vs baseline: 1.3416x; 1.2585x over previous
"""LPKT knowledge-tracing kernel for 8x Trainium2 NeuronCores.

Data-parallel over batch: B=32 -> 4 batches per core. Per core the recurrent
state h [4, C=256, K=128] lives in SBUF as hT [K=128 partitions, (b,c)=1024
free] in bf16.  All matmuls are bf16 (1 HW pass + 1 cyc/row vs fp32's 2
passes at 4 cyc/row, and ~80ns LDWEIGHTS instead of ~440ns).  All
elementwise work is on DVE: gamma*h uses tensor_tensor (which the compiler
runs in the 2x bf16 mode, ~200ns per [128,256]); h_new and the h_tilde
accumulation need scalar_tensor_tensor (per-batch scalar / accum_out),
which only runs at 1x.  GpSimd is deliberately idle: it shares SBUF ports
with DVE, so offloading bulk elementwise there is net negative.

q rows are broadcast across all 128 partitions by DMA (stride-0 source via
AP.partition_broadcast), one 16-step window (4MB) at a time, double
buffered -- the descriptors fan out over all 16 DMA engines, so no compute
engine spends cycles on replication.

The 4 batches are processed as TWO independent 2-batch streams per step so
stream B's gate chain (PE matmul -> sigmoid -> LG -> W4b matmul -> u)
overlaps stream A's state-update tail.  h_tilde columns accumulate into a
persistent HT_all buffer [K, 4*(T+1)]; gate matmuls read their [K,2] slice
directly and the y head (W5 + sigmoid + reduce) runs once, batched, after
the loop.
"""

import numpy as np

B, S = 32, 128
NUM_Q, NUM_C = 10000, 256
K = 128
C = NUM_C
NCORES = 8
BL = B // NCORES  # 4 batches per core
T = S - 1  # 127 recurrence steps
QW = 16  # q broadcast window, steps
NWIN = S // QW

_cache = {}


def _build():
    import concourse.bass as bass  # noqa: F401
    import concourse.mybir as mybir
    import concourse.tile as tile
    from concourse import bacc

    fp32 = mybir.dt.float32
    bf16 = mybir.dt.bfloat16
    AF = mybir.ActivationFunctionType
    OP = mybir.AluOpType

    nc = bacc.Bacc()

    # ---------------- DRAM I/O ----------------
    d = {}

    def din(name, shape, dt_=bf16):
        t = nc.dram_tensor(name, shape, dt_, kind="ExternalInput")
        d[name] = t
        return t

    din("eT", [K, S * BL])       # e_emb^T, free layout (s, b) s-major
    din("atT", [K, S * BL])
    din("itT", [K, S * BL])
    din("a_row", [1, S * BL])
    din("h0T4", [K, BL * C])
    q_dram = nc.dram_tensor("qD", [S, BL * C], bf16, kind="ExternalInput")
    for w in ["W1a", "W1b", "W2a2", "W2b2", "W2c2", "W2d2",
              "W3a", "W3b", "W3c", "W3d", "W4a", "W4b", "W4c",
              "W5a", "W5b"]:
        din(w, [K, K])
    for w in ["w1c", "b1r", "b2r2", "b3r", "b4r", "b5r"]:
        din(w, [1, K])
    din("ones512", [1, 512])
    din("I128", [K, K])
    din("ones128c", [K, 1])
    y_dram = nc.dram_tensor("y_out", [1, BL * T], fp32, kind="ExternalOutput")

    from contextlib import ExitStack

    with tile.TileContext(nc) as tc, ExitStack() as ctx:
        singles = ctx.enter_context(tc.tile_pool(name="singles", bufs=1))
        state = ctx.enter_context(tc.tile_pool(name="state", bufs=1))
        sm = ctx.enter_context(tc.tile_pool(name="sm", bufs=3))
        qw = ctx.enter_context(tc.tile_pool(name="qw", bufs=2))
        pp = ctx.enter_context(tc.tile_pool(name="pp", bufs=2, space="PSUM"))
        psm = ctx.enter_context(tc.tile_pool(name="psm", bufs=2, space="PSUM"))

        # ---------------- load everything to SBUF ----------------
        sb = {}
        for name, dt_ in d.items():
            if name == "h0T4":
                continue  # loaded straight into the state tile below
            t_ = singles.tile(list(dt_.shape), dt_.dtype, tag=name)
            nc.sync.dma_start(out=t_[:], in_=dt_[:])
            sb[name] = t_

        # recurrent state h, DMA'd straight from the prepped h0 tile
        s_h = state.tile([K, BL * C], bf16, tag="h")
        nc.sync.dma_start(out=s_h[:], in_=d["h0T4"][:])

        # q windows: all 128 partitions get a copy of q rows [16w, 16w+16)
        qwin = [None] * NWIN

        def qwin_load(w):
            wt = qw.tile([K, QW * BL * C], bf16, tag="qwin")
            src = q_dram[w * QW:(w + 1) * QW, :].partition_broadcast(K)
            nc.sync.dma_start(out=wt[:], in_=src)
            qwin[w] = wt

        qwin_load(0)
        qwin_load(1)

        def qsl(t):
            # [K, 1024] replicated q row for step t
            base = (t % QW) * BL * C
            return qwin[t // QW][:, base:base + BL * C]

        # collapse the ~30 input-DMA dependencies
        tc.strict_bb_all_engine_barrier()

        s_gam = state.tile([K, BL * C], bf16, tag="gam")
        s_m = state.tile([K, BL * C], bf16, tag="m")
        # h_tilde history: block t (cols 4t:4t+4) = h_tilde at step t
        s_HT = state.tile([K, (T + 1) * BL], bf16, tag="HT")

        # ---------------- precompute: allT, Z23, U4, Y5 ----------------
        p_all = pp.tile([K, 512], fp32, tag="pbig", bufs=1)
        nc.tensor.matmul(out=p_all[:], lhsT=sb["W1a"][:], rhs=sb["eT"][:],
                         start=True, stop=False)
        nc.tensor.matmul(out=p_all[:], lhsT=sb["W1b"][:], rhs=sb["atT"][:],
                         start=False, stop=False)
        nc.tensor.matmul(out=p_all[:], lhsT=sb["w1c"][:], rhs=sb["a_row"][:],
                         start=False, stop=False)
        nc.tensor.matmul(out=p_all[:], lhsT=sb["b1r"][:],
                         rhs=sb["ones512"][:], start=False, stop=True)
        s_allT = singles.tile([K, 512], bf16, tag="allT")
        nc.vector.tensor_copy(out=s_allT[:], in_=p_all[:])

        # Z23[k, t, g, b2, b]: gate g in {2,3}, stream b2, batch-in-stream b
        s_Z23 = singles.tile([K, T, 2, 2, 2], bf16, tag="Z23")

        def precompute_z(Wpre, Wit, Wlearn, brow, g):
            ptile = pp.tile([K, T * BL], fp32, tag="pbig", bufs=1)
            nc.tensor.matmul(out=ptile[:], lhsT=sb[Wit][:],
                             rhs=sb["itT"][:, 0:T * BL], start=True, stop=False)
            nc.tensor.matmul(out=ptile[:, BL:T * BL], lhsT=sb[Wpre][:],
                             rhs=s_allT[:, 0:(T - 1) * BL],
                             start=False, stop=False, skip_group_check=True)
            nc.tensor.matmul(out=ptile[:], lhsT=sb[Wlearn][:],
                             rhs=s_allT[:, 0:T * BL], start=False, stop=False)
            nc.tensor.matmul(out=ptile[:], lhsT=sb[brow][:],
                             rhs=sb["ones512"][:, 0:T * BL], start=False,
                             stop=True)
            nc.vector.tensor_copy(
                out=s_Z23[:, :, g, :, :],
                in_=ptile[:].rearrange("k (t b2 b) -> k t b2 b", b2=2, b=2))

        precompute_z("W2a2", "W2b2", "W2c2", "b2r2", 0)
        precompute_z("W3a", "W3b", "W3c", "b3r", 1)

        # U4[k, (t,b)] = it@W4c + b4
        p_u4 = pp.tile([K, T * BL], fp32, tag="pbig", bufs=1)
        nc.tensor.matmul(out=p_u4[:], lhsT=sb["W4c"][:],
                         rhs=sb["itT"][:, 0:T * BL], start=True, stop=False)
        nc.tensor.matmul(out=p_u4[:], lhsT=sb["b4r"][:],
                         rhs=sb["ones512"][:, 0:T * BL], start=False, stop=True)
        s_U4 = singles.tile([K, T * BL], bf16, tag="U4")
        nc.vector.tensor_copy(out=s_U4[:], in_=p_u4[:])

        # Y5[k, (t,b)] = e_emb[t+1]@W5a + b5
        p_y5 = pp.tile([K, T * BL], fp32, tag="pbig", bufs=1)
        nc.tensor.matmul(out=p_y5[:], lhsT=sb["W5a"][:],
                         rhs=sb["eT"][:, BL:S * BL], start=True, stop=False)
        nc.tensor.matmul(out=p_y5[:], lhsT=sb["b5r"][:],
                         rhs=sb["ones512"][:, 0:T * BL], start=False, stop=True)
        s_Y5 = singles.tile([K, T * BL], fp32, tag="Y5")
        nc.vector.tensor_copy(out=s_Y5[:], in_=p_y5[:])

        # ---------------- h_tilde init (with q_0) ----------------
        for b in range(BL):
            cs = slice(b * C, (b + 1) * C)
            nc.vector.scalar_tensor_tensor(
                out=s_m[:, cs], in0=s_h[:, cs], scalar=0.0,
                in1=qsl(0)[:, cs], op0=OP.bypass, op1=OP.mult,
                accum_out=s_HT[:, b:b + 1])

        # ---------------- the recurrence (two 2-batch streams) ----------
        for t in range(T):
            if t % QW == 0 and t > 0 and (t // QW + 1) < NWIN:
                qwin_load(t // QW + 1)

            ps = psm.tile([K, 16], fp32, tag="small")
            for s2 in range(2):
                o = s2 * 8
                ht_sl = s_HT[:, t * BL + 2 * s2:t * BL + 2 * s2 + 2]
                nc.tensor.matmul(out=ps[:, o:o + 2], lhsT=sb["W2d2"][:],
                                 rhs=ht_sl, start=True, stop=False)
                nc.tensor.matmul(out=ps[:, o + 2:o + 4], lhsT=sb["W3d"][:],
                                 rhs=ht_sl, start=True, stop=False)
                # += Z23[t] on PE (identity pass-through), so no DVE add sits
                # on the gate-critical path
                nc.tensor.matmul(out=ps[:, o:o + 4], lhsT=sb["I128"][:],
                                 rhs=s_Z23[:, t, :, s2, :], start=False,
                                 stop=True, skip_group_check=True)
                # gamma_f preact for this stream's two batches
                pPs = pp.tile([K, 512], fp32, tag=f"pP{s2}")
                nc.tensor.matmul(out=pPs[:], lhsT=sb["W4a"][:],
                                 rhs=s_h[:, s2 * 512:(s2 + 1) * 512],
                                 start=True, stop=True)

                s23 = sm.tile([K, 4], bf16, tag=f"s23{s2}")
                nc.scalar.activation(out=s23[:], in_=ps[:, o:o + 4],
                                     func=AF.Sigmoid)
                # LG on the otherwise-idle GpSimd queue: never waits behind
                # DVE bulk work
                LGT = sm.tile([K, 2], bf16, tag=f"LGT{s2}")
                nc.gpsimd.tensor_mul(out=LGT[:], in0=s23[:, 0:2],
                                     in1=s23[:, 2:4])

                # u = LG @ W4b + U4[t] (U4 added on PE); psum -> SBUF move on
                # ACT, same queue as the gamma sigmoids that consume it
                nc.tensor.matmul(out=ps[:, o + 4:o + 6], lhsT=sb["W4b"][:],
                                 rhs=LGT[:], start=True, stop=False)
                nc.tensor.matmul(
                    out=ps[:, o + 4:o + 6], lhsT=sb["I128"][:],
                    rhs=s_U4[:, t * BL + 2 * s2:t * BL + 2 * s2 + 2],
                    start=False, stop=True)
                uT = sm.tile([K, 2], fp32, tag=f"uT{s2}")
                nc.scalar.copy(out=uT[:], in_=ps[:, o + 4:o + 6])

                for b in range(2):
                    gb = 2 * s2 + b
                    cs = slice(gb * C, (gb + 1) * C)
                    nc.scalar.activation(out=s_gam[:, cs],
                                         in_=pPs[:, b * C:(b + 1) * C],
                                         func=AF.Sigmoid, bias=uT[:, b:b + 1])
                    # m = gamma * h
                    nc.vector.tensor_mul(out=s_m[:, cs], in0=s_gam[:, cs],
                                         in1=s_h[:, cs])
                    # h_new = q_e * LG + m
                    nc.vector.scalar_tensor_tensor(
                        out=s_h[:, cs], in0=qsl(t)[:, cs],
                        scalar=LGT[:, b:b + 1], in1=s_m[:, cs],
                        op0=OP.mult, op1=OP.add)
                    # h_tilde accumulation with q_{t+1}
                    col = (t + 1) * BL + gb
                    nc.vector.scalar_tensor_tensor(
                        out=s_m[:, cs], in0=s_h[:, cs], scalar=0.0,
                        in1=qsl(t + 1)[:, cs], op0=OP.bypass, op1=OP.mult,
                        accum_out=s_HT[:, col:col + 1])

        # ---------------- y head, batched over all steps ----------------
        p_y = pp.tile([K, T * BL], fp32, tag="pbig", bufs=1)
        nc.tensor.matmul(out=p_y[:], lhsT=sb["W5b"][:],
                         rhs=s_HT[:, BL:(T + 1) * BL], start=True, stop=True)
        tY = singles.tile([K, T * BL], fp32, tag="tY")
        nc.vector.tensor_add(out=tY[:], in0=p_y[:], in1=s_Y5[:])
        sY = singles.tile([K, T * BL], bf16, tag="sY")
        nc.scalar.activation(out=sY[:], in_=tY[:], func=AF.Sigmoid)
        p_ys = psm.tile([1, T * BL], fp32, tag="yacc", bufs=1)
        nc.tensor.matmul(out=p_ys[:], lhsT=sb["ones128c"][:], rhs=sY[:],
                         start=True, stop=True)
        s_y = singles.tile([1, T * BL], fp32, tag="yout")
        nc.vector.tensor_copy(out=s_y[:], in_=p_ys[:])
        nc.sync.dma_start(out=y_dram[:], in_=s_y[:])

    nc.compile()
    return nc


def _prep_inputs(inputs):
    """Host-side sharding + layout prep. Returns per-core input dicts."""
    import ml_dtypes

    bf = ml_dtypes.bfloat16
    f32 = np.float32
    e_idx = np.asarray(inputs["e_data"]).astype(np.int64)
    at_idx = np.asarray(inputs["at_data"]).astype(np.int64)
    it_idx = np.asarray(inputs["it_data"]).astype(np.int64)
    a_data = np.asarray(inputs["a_data"], dtype=f32)
    q_matrix = np.asarray(inputs["q_matrix"], dtype=f32)
    e_E = np.asarray(inputs["e_E"], dtype=bf)
    at_E = np.asarray(inputs["at_E"], dtype=bf)
    it_E = np.asarray(inputs["it_E"], dtype=bf)
    W1 = np.asarray(inputs["W1"], dtype=f32)
    W2 = np.asarray(inputs["W2"], dtype=f32)
    W3 = np.asarray(inputs["W3"], dtype=f32)
    W4 = np.asarray(inputs["W4"], dtype=f32)
    W5 = np.asarray(inputs["W5"], dtype=f32)
    h0 = np.asarray(inputs["h0"], dtype=f32)

    def bfc(x):
        return np.ascontiguousarray(np.asarray(x, dtype=bf))

    shared = {
        "W1a": bfc(W1[0:K]), "W1b": bfc(W1[K:2 * K]),
        "w1c": bfc(W1[2 * K:].sum(0)[None, :]),
        "b1r": bfc(np.asarray(inputs["b1"], dtype=f32)[None, :]),
        "W2a2": bfc(2 * W2[0:K]), "W2b2": bfc(2 * W2[K:2 * K]),
        "W2c2": bfc(2 * W2[2 * K:3 * K]), "W2d2": bfc(2 * W2[3 * K:]),
        "b2r2": bfc(2 * np.asarray(inputs["b2"], dtype=f32)[None, :]),
        "W3a": bfc(W3[0:K]), "W3b": bfc(W3[K:2 * K]),
        "W3c": bfc(W3[2 * K:3 * K]), "W3d": bfc(W3[3 * K:]),
        "b3r": bfc(np.asarray(inputs["b3"], dtype=f32)[None, :]),
        "W4a": bfc(W4[0:K]), "W4b": bfc(W4[K:2 * K]), "W4c": bfc(W4[2 * K:]),
        "b4r": bfc(np.asarray(inputs["b4"], dtype=f32)[None, :]),
        "W5a": bfc(W5[0:K]), "W5b": bfc(W5[K:]),
        "b5r": bfc(np.asarray(inputs["b5"], dtype=f32)[None, :]),
        "ones512": bfc(np.ones((1, 512), f32)),
        "ones128c": bfc(np.ones((K, 1), f32)),
        "I128": bfc(np.eye(K, dtype=f32)),
        "h0T4": bfc(np.tile(np.ascontiguousarray(h0.T), (1, BL))),
    }

    in_maps = []
    for g in range(NCORES):
        bg = slice(g * BL, (g + 1) * BL)
        e_emb = e_E[e_idx[bg]]          # [4, S, K] bf16
        at_emb = at_E[at_idx[bg]]
        it_emb = it_E[it_idx[bg]]
        q_all = q_matrix[e_idx[bg]]     # [4, S, C] f32
        m = dict(shared)
        # [K, (s, b)] s-major layouts
        m["eT"] = bfc(e_emb.transpose(2, 1, 0).reshape(K, S * BL))
        m["atT"] = bfc(at_emb.transpose(2, 1, 0).reshape(K, S * BL))
        m["itT"] = bfc(it_emb.transpose(2, 1, 0).reshape(K, S * BL))
        m["qD"] = bfc(q_all.transpose(1, 0, 2).reshape(S, BL * C))
        m["a_row"] = bfc(a_data[bg].T.reshape(1, S * BL))
        in_maps.append(m)
    return in_maps


def _run(inputs, trace=False):
    from concourse.bass_utils import run_bass_kernel_spmd

    if "nc" not in _cache:
        _cache["nc"] = _build()
    nc = _cache["nc"]
    in_maps = _prep_inputs(inputs)
    res = run_bass_kernel_spmd(nc, in_maps, core_ids=list(range(NCORES)),
                               trace=trace)
    pred = np.zeros((B, S), np.float32)
    for g in range(NCORES):
        y = res.results[g]["y_out"].reshape(T, BL)  # [t, b]
        pred[g * BL:(g + 1) * BL, 1:] = y.T / K
    return pred, res


def kernel(**inputs):
    return _run(inputs)[0]


# revision 18
# speedup vs baseline: 1.3657x; 1.0180x over previous
"""LPKT knowledge-tracing kernel for 8x Trainium2 NeuronCores.

Data-parallel over batch: B=32 -> 4 batches per core. Per core the recurrent
state h [4, C=256, K=128] lives in SBUF as hT [K=128 partitions, (b,c)=1024
free] in bf16.  All matmuls are bf16 (1 HW pass + 1 cyc/row vs fp32's 2
passes at 4 cyc/row, and ~80ns LDWEIGHTS instead of ~440ns).

Bulk elementwise work is on DVE: gamma*h uses tensor_tensor (2x bf16 DVE
mode, ~250ns per [128,256]); h_new and the h_tilde accumulation need
scalar_tensor_tensor (per-batch scalar / accum_out), which only runs 1x.
The gate-critical small ops are kept OFF the congested DVE queue so they
never wait behind bulk work: the Z23/U4 constant adds ride the PSUM
accumulation groups as I128 pass-through matmuls on PE, the u psum->SBUF
move is an ACT copy (same queue as the gamma sigmoids that consume it),
and the tiny LG product runs on the otherwise-idle GpSimd.  (Bulk work on
GpSimd is net negative -- it shares SBUF ports with DVE.)

q rows are broadcast across all 128 partitions by DMA (stride-0 source via
AP.partition_broadcast), one 16-step window (4MB) at a time, double
buffered -- the descriptors fan out over all 16 DMA engines, so no compute
engine spends cycles on replication.

The 4 batches are processed as TWO independent 2-batch streams per step so
stream B's gate chain (PE matmul -> sigmoid -> LG -> W4b matmul -> u)
overlaps stream A's state-update tail.  Emission order is deliberately
per-stream sequential: the Tile scheduler follows it closely, and
cross-stream reorderings (hoisting gate matmuls, interleaving tails)
measurably regress.  h_tilde columns accumulate into a persistent HT_all
buffer [K, 4*(T+1)]; gate matmuls read their [K,2] slice directly and the
y head (W5 + sigmoid + reduce) runs once, batched, after the loop.
"""

import numpy as np

B, S = 32, 128
NUM_Q, NUM_C = 10000, 256
K = 128
C = NUM_C
NCORES = 8
BL = B // NCORES  # 4 batches per core
T = S - 1  # 127 recurrence steps
QW = 16  # q broadcast window, steps
NWIN = S // QW

_cache = {}


def _build():
    import concourse.bass as bass  # noqa: F401
    import concourse.mybir as mybir
    import concourse.tile as tile
    from concourse import bacc

    fp32 = mybir.dt.float32
    bf16 = mybir.dt.bfloat16
    AF = mybir.ActivationFunctionType
    OP = mybir.AluOpType

    nc = bacc.Bacc()

    # ---------------- DRAM I/O ----------------
    d = {}

    def din(name, shape, dt_=bf16):
        t = nc.dram_tensor(name, shape, dt_, kind="ExternalInput")
        d[name] = t
        return t

    din("eT", [K, S * BL])       # e_emb^T, free layout (s, b) s-major
    din("atT", [K, S * BL])
    din("itT", [K, S * BL])
    din("a_row", [1, S * BL])
    din("h0T4", [K, BL * C])
    q_dram = nc.dram_tensor("qD", [S, BL * C], bf16, kind="ExternalInput")
    for w in ["W1a", "W1b", "W2a2", "W2b2", "W2c2", "W2d2",
              "W3a", "W3b", "W3c", "W3d", "W4a", "W4b", "W4c",
              "W5a", "W5b"]:
        din(w, [K, K])
    for w in ["w1c", "b1r", "b2r2", "b3r", "b4r", "b5r"]:
        din(w, [1, K])
    din("ones512", [1, 512])
    din("dD", [1, T * BL])
    din("I128", [K, K])
    din("ones128c", [K, 1])
    y_dram = nc.dram_tensor("y_out", [1, BL * T], fp32, kind="ExternalOutput")

    from contextlib import ExitStack

    with tile.TileContext(nc) as tc, ExitStack() as ctx:
        singles = ctx.enter_context(tc.tile_pool(name="singles", bufs=1))
        state = ctx.enter_context(tc.tile_pool(name="state", bufs=1))
        sm = ctx.enter_context(tc.tile_pool(name="sm", bufs=3))
        qw = ctx.enter_context(tc.tile_pool(name="qw", bufs=2))
        pp = ctx.enter_context(tc.tile_pool(name="pp", bufs=2, space="PSUM"))
        psm = ctx.enter_context(tc.tile_pool(name="psm", bufs=2, space="PSUM"))

        # ---------------- load everything to SBUF ----------------
        sb = {}
        for name, dt_ in d.items():
            if name == "h0T4":
                continue  # loaded straight into the state tile below
            t_ = singles.tile(list(dt_.shape), dt_.dtype, tag=name)
            nc.sync.dma_start(out=t_[:], in_=dt_[:])
            sb[name] = t_

        # recurrent state h, DMA'd straight from the prepped h0 tile
        s_h = state.tile([K, BL * C], bf16, tag="h")
        nc.sync.dma_start(out=s_h[:], in_=d["h0T4"][:])

        # q windows: all 128 partitions get a copy of q rows [16w, 16w+16)
        qwin = [None] * NWIN

        def qwin_load(w):
            wt = qw.tile([K, QW * BL * C], bf16, tag="qwin")
            src = q_dram[w * QW:(w + 1) * QW, :].partition_broadcast(K)
            nc.sync.dma_start(out=wt[:], in_=src)
            qwin[w] = wt

        qwin_load(0)
        qwin_load(1)

        # d[t,b] = q_t . q_{t+1} broadcast to all partitions (tiny, one DMA)
        s_d = state.tile([K, T * BL], bf16, tag="dT")
        nc.sync.dma_start(out=s_d[:],
                          in_=d["dD"][:].partition_broadcast(K))

        def qsl(t):
            # [K, 1024] replicated q row for step t
            base = (t % QW) * BL * C
            return qwin[t // QW][:, base:base + BL * C]

        # collapse the ~30 input-DMA dependencies
        tc.strict_bb_all_engine_barrier()

        s_gam = state.tile([K, BL * C], bf16, tag="gam")
        s_m = state.tile([K, BL * C], bf16, tag="m")
        # h_tilde history: block t (cols 4t:4t+4) = h_tilde at step t
        s_HT = state.tile([K, (T + 1) * BL], bf16, tag="HT")

        # ---------------- precompute: allT, Z23, U4, Y5 ----------------
        p_all = pp.tile([K, 512], fp32, tag="pbig", bufs=1)
        nc.tensor.matmul(out=p_all[:], lhsT=sb["W1a"][:], rhs=sb["eT"][:],
                         start=True, stop=False)
        nc.tensor.matmul(out=p_all[:], lhsT=sb["W1b"][:], rhs=sb["atT"][:],
                         start=False, stop=False)
        nc.tensor.matmul(out=p_all[:], lhsT=sb["w1c"][:], rhs=sb["a_row"][:],
                         start=False, stop=False)
        nc.tensor.matmul(out=p_all[:], lhsT=sb["b1r"][:],
                         rhs=sb["ones512"][:], start=False, stop=True)
        s_allT = singles.tile([K, 512], bf16, tag="allT")
        nc.vector.tensor_copy(out=s_allT[:], in_=p_all[:])

        # Z23[k, t, g, b2, b]: gate g in {2,3}, stream b2, batch-in-stream b
        s_Z23 = singles.tile([K, T, 2, 2, 2], bf16, tag="Z23")

        def precompute_z(Wpre, Wit, Wlearn, brow, g):
            ptile = pp.tile([K, T * BL], fp32, tag="pbig", bufs=1)
            nc.tensor.matmul(out=ptile[:], lhsT=sb[Wit][:],
                             rhs=sb["itT"][:, 0:T * BL], start=True, stop=False)
            nc.tensor.matmul(out=ptile[:, BL:T * BL], lhsT=sb[Wpre][:],
                             rhs=s_allT[:, 0:(T - 1) * BL],
                             start=False, stop=False, skip_group_check=True)
            nc.tensor.matmul(out=ptile[:], lhsT=sb[Wlearn][:],
                             rhs=s_allT[:, 0:T * BL], start=False, stop=False)
            nc.tensor.matmul(out=ptile[:], lhsT=sb[brow][:],
                             rhs=sb["ones512"][:, 0:T * BL], start=False,
                             stop=True)
            nc.vector.tensor_copy(
                out=s_Z23[:, :, g, :, :],
                in_=ptile[:].rearrange("k (t b2 b) -> k t b2 b", b2=2, b=2))

        precompute_z("W2a2", "W2b2", "W2c2", "b2r2", 0)
        precompute_z("W3a", "W3b", "W3c", "b3r", 1)

        # U4[k, (t,b)] = it@W4c + b4
        p_u4 = pp.tile([K, T * BL], fp32, tag="pbig", bufs=1)
        nc.tensor.matmul(out=p_u4[:], lhsT=sb["W4c"][:],
                         rhs=sb["itT"][:, 0:T * BL], start=True, stop=False)
        nc.tensor.matmul(out=p_u4[:], lhsT=sb["b4r"][:],
                         rhs=sb["ones512"][:, 0:T * BL], start=False, stop=True)
        s_U4 = singles.tile([K, T * BL], bf16, tag="U4")
        nc.vector.tensor_copy(out=s_U4[:], in_=p_u4[:])

        # Y5[k, (t,b)] = e_emb[t+1]@W5a + b5
        p_y5 = pp.tile([K, T * BL], fp32, tag="pbig", bufs=1)
        nc.tensor.matmul(out=p_y5[:], lhsT=sb["W5a"][:],
                         rhs=sb["eT"][:, BL:S * BL], start=True, stop=False)
        nc.tensor.matmul(out=p_y5[:], lhsT=sb["b5r"][:],
                         rhs=sb["ones512"][:, 0:T * BL], start=False, stop=True)
        s_Y5 = singles.tile([K, T * BL], fp32, tag="Y5")
        nc.vector.tensor_copy(out=s_Y5[:], in_=p_y5[:])

        # ---------------- h_tilde init (with q_0) ----------------
        for b in range(BL):
            cs = slice(b * C, (b + 1) * C)
            nc.vector.scalar_tensor_tensor(
                out=s_m[:, cs], in0=s_h[:, cs], scalar=0.0,
                in1=qsl(0)[:, cs], op0=OP.bypass, op1=OP.mult,
                accum_out=s_HT[:, b:b + 1])

        # ---------------- the recurrence (two 2-batch streams) ----------
        for t in range(T):
            if t % QW == 0 and t > 0 and (t // QW + 1) < NWIN:
                qwin_load(t // QW + 1)

            ps = psm.tile([K, 16], fp32, tag="small")
            for s2 in range(2):
                o = s2 * 8
                ht_sl = s_HT[:, t * BL + 2 * s2:t * BL + 2 * s2 + 2]
                nc.tensor.matmul(out=ps[:, o:o + 2], lhsT=sb["W2d2"][:],
                                 rhs=ht_sl, start=True, stop=False)
                nc.tensor.matmul(out=ps[:, o + 2:o + 4], lhsT=sb["W3d"][:],
                                 rhs=ht_sl, start=True, stop=False)
                # += Z23[t] on PE (identity pass-through), so no DVE add sits
                # on the gate-critical path
                nc.tensor.matmul(out=ps[:, o:o + 4], lhsT=sb["I128"][:],
                                 rhs=s_Z23[:, t, :, s2, :], start=False,
                                 stop=True, skip_group_check=True)
                # gamma_f preact for this stream's two batches
                pPs = pp.tile([K, 512], fp32, tag=f"pP{s2}")
                nc.tensor.matmul(out=pPs[:], lhsT=sb["W4a"][:],
                                 rhs=s_h[:, s2 * 512:(s2 + 1) * 512],
                                 start=True, stop=True)

                s23 = sm.tile([K, 4], bf16, tag=f"s23{s2}")
                nc.scalar.activation(out=s23[:], in_=ps[:, o:o + 4],
                                     func=AF.Sigmoid)
                # LG on the otherwise-idle GpSimd queue: never waits behind
                # DVE bulk work
                LGT = sm.tile([K, 2], bf16, tag=f"LGT{s2}")
                nc.gpsimd.tensor_mul(out=LGT[:], in0=s23[:, 0:2],
                                     in1=s23[:, 2:4])
                t1 = sm.tile([K, 2], bf16, tag=f"t1{s2}")
                nc.gpsimd.tensor_mul(
                    out=t1[:], in0=LGT[:],
                    in1=s_d[:, t * BL + 2 * s2:t * BL + 2 * s2 + 2])

                # u = LG @ W4b + U4[t] (U4 added on PE); psum -> SBUF move on
                # ACT, same queue as the gamma sigmoids that consume it
                nc.tensor.matmul(out=ps[:, o + 4:o + 6], lhsT=sb["W4b"][:],
                                 rhs=LGT[:], start=True, stop=False)
                nc.tensor.matmul(
                    out=ps[:, o + 4:o + 6], lhsT=sb["I128"][:],
                    rhs=s_U4[:, t * BL + 2 * s2:t * BL + 2 * s2 + 2],
                    start=False, stop=True)
                uT = sm.tile([K, 2], fp32, tag=f"uT{s2}")
                nc.scalar.copy(out=uT[:], in_=ps[:, o + 4:o + 6])

                tmp = sm.tile([K, 2], fp32, tag=f"htmp{s2}")
                for b in range(2):
                    gb = 2 * s2 + b
                    cs = slice(gb * C, (gb + 1) * C)
                    nc.scalar.activation(out=s_gam[:, cs],
                                         in_=pPs[:, b * C:(b + 1) * C],
                                         func=AF.Sigmoid, bias=uT[:, b:b + 1])
                    # m = gamma * h
                    nc.vector.tensor_mul(out=s_m[:, cs], in0=s_gam[:, cs],
                                         in1=s_h[:, cs])
                    # sum_c q_{t+1}*m (h_tilde minus the d*LG correction,
                    # so it does not wait for h_new)
                    nc.vector.scalar_tensor_tensor(
                        out=s_gam[:, cs], in0=s_m[:, cs], scalar=0.0,
                        in1=qsl(t + 1)[:, cs], op0=OP.bypass, op1=OP.mult,
                        accum_out=tmp[:, b:b + 1])
                # h_tilde = d*LG + sum_c q_{t+1}*m  (back-to-back on DVE)
                col0 = (t + 1) * BL + 2 * s2
                nc.vector.tensor_add(out=s_HT[:, col0:col0 + 2],
                                     in0=tmp[:], in1=t1[:])
                for b in range(2):
                    gb = 2 * s2 + b
                    cs = slice(gb * C, (gb + 1) * C)
                    # h_new = q_e * LG + m (only needed by next step's W4a@h)
                    nc.vector.scalar_tensor_tensor(
                        out=s_h[:, cs], in0=qsl(t)[:, cs],
                        scalar=LGT[:, b:b + 1], in1=s_m[:, cs],
                        op0=OP.mult, op1=OP.add)

        # ---------------- y head, batched over all steps ----------------
        p_y = pp.tile([K, T * BL], fp32, tag="pbig", bufs=1)
        nc.tensor.matmul(out=p_y[:], lhsT=sb["W5b"][:],
                         rhs=s_HT[:, BL:(T + 1) * BL], start=True, stop=True)
        tY = singles.tile([K, T * BL], fp32, tag="tY")
        nc.vector.tensor_add(out=tY[:], in0=p_y[:], in1=s_Y5[:])
        sY = singles.tile([K, T * BL], bf16, tag="sY")
        nc.scalar.activation(out=sY[:], in_=tY[:], func=AF.Sigmoid)
        p_ys = psm.tile([1, T * BL], fp32, tag="yacc", bufs=1)
        nc.tensor.matmul(out=p_ys[:], lhsT=sb["ones128c"][:], rhs=sY[:],
                         start=True, stop=True)
        s_y = singles.tile([1, T * BL], fp32, tag="yout")
        nc.vector.tensor_copy(out=s_y[:], in_=p_ys[:])
        nc.sync.dma_start(out=y_dram[:], in_=s_y[:])

    nc.compile()
    return nc


def _prep_inputs(inputs):
    """Host-side sharding + layout prep. Returns per-core input dicts."""
    import ml_dtypes

    bf = ml_dtypes.bfloat16
    f32 = np.float32
    e_idx = np.asarray(inputs["e_data"]).astype(np.int64)
    at_idx = np.asarray(inputs["at_data"]).astype(np.int64)
    it_idx = np.asarray(inputs["it_data"]).astype(np.int64)
    a_data = np.asarray(inputs["a_data"], dtype=f32)
    q_matrix = np.asarray(inputs["q_matrix"], dtype=f32)
    e_E = np.asarray(inputs["e_E"], dtype=bf)
    at_E = np.asarray(inputs["at_E"], dtype=bf)
    it_E = np.asarray(inputs["it_E"], dtype=bf)
    W1 = np.asarray(inputs["W1"], dtype=f32)
    W2 = np.asarray(inputs["W2"], dtype=f32)
    W3 = np.asarray(inputs["W3"], dtype=f32)
    W4 = np.asarray(inputs["W4"], dtype=f32)
    W5 = np.asarray(inputs["W5"], dtype=f32)
    h0 = np.asarray(inputs["h0"], dtype=f32)

    def bfc(x):
        return np.ascontiguousarray(np.asarray(x, dtype=bf))

    shared = {
        "W1a": bfc(W1[0:K]), "W1b": bfc(W1[K:2 * K]),
        "w1c": bfc(W1[2 * K:].sum(0)[None, :]),
        "b1r": bfc(np.asarray(inputs["b1"], dtype=f32)[None, :]),
        "W2a2": bfc(2 * W2[0:K]), "W2b2": bfc(2 * W2[K:2 * K]),
        "W2c2": bfc(2 * W2[2 * K:3 * K]), "W2d2": bfc(2 * W2[3 * K:]),
        "b2r2": bfc(2 * np.asarray(inputs["b2"], dtype=f32)[None, :]),
        "W3a": bfc(W3[0:K]), "W3b": bfc(W3[K:2 * K]),
        "W3c": bfc(W3[2 * K:3 * K]), "W3d": bfc(W3[3 * K:]),
        "b3r": bfc(np.asarray(inputs["b3"], dtype=f32)[None, :]),
        "W4a": bfc(W4[0:K]), "W4b": bfc(W4[K:2 * K]), "W4c": bfc(W4[2 * K:]),
        "b4r": bfc(np.asarray(inputs["b4"], dtype=f32)[None, :]),
        "W5a": bfc(W5[0:K]), "W5b": bfc(W5[K:]),
        "b5r": bfc(np.asarray(inputs["b5"], dtype=f32)[None, :]),
        "ones512": bfc(np.ones((1, 512), f32)),
        "ones128c": bfc(np.ones((K, 1), f32)),
        "I128": bfc(np.eye(K, dtype=f32)),
        "h0T4": bfc(np.tile(np.ascontiguousarray(h0.T), (1, BL))),
    }

    in_maps = []
    for g in range(NCORES):
        bg = slice(g * BL, (g + 1) * BL)
        e_emb = e_E[e_idx[bg]]          # [4, S, K] bf16
        at_emb = at_E[at_idx[bg]]
        it_emb = it_E[it_idx[bg]]
        q_all = q_matrix[e_idx[bg]]     # [4, S, C] f32
        m = dict(shared)
        # [K, (s, b)] s-major layouts
        m["eT"] = bfc(e_emb.transpose(2, 1, 0).reshape(K, S * BL))
        m["atT"] = bfc(at_emb.transpose(2, 1, 0).reshape(K, S * BL))
        m["itT"] = bfc(it_emb.transpose(2, 1, 0).reshape(K, S * BL))
        m["qD"] = bfc(q_all.transpose(1, 0, 2).reshape(S, BL * C))
        dmat = (q_all[:, :-1] * q_all[:, 1:]).sum(-1)  # [BL, T]
        m["dD"] = bfc(dmat.T.reshape(1, T * BL))
        m["a_row"] = bfc(a_data[bg].T.reshape(1, S * BL))
        in_maps.append(m)
    return in_maps


def _run(inputs, trace=False):
    from concourse.bass_utils import run_bass_kernel_spmd

    if "nc" not in _cache:
        _cache["nc"] = _build()
    nc = _cache["nc"]
    in_maps = _prep_inputs(inputs)
    res = run_bass_kernel_spmd(nc, in_maps, core_ids=list(range(NCORES)),
                               trace=trace)
    pred = np.zeros((B, S), np.float32)
    for g in range(NCORES):
        y = res.results[g]["y_out"].reshape(T, BL)  # [t, b]
        pred[g * BL:(g + 1) * BL, 1:] = y.T / K
    return pred, res


def kernel(**inputs):
    return _run(inputs)[0]


# revision 20
# speedup vs baseline: 1.3737x; 1.0058x over previous
"""LPKT knowledge-tracing kernel for 8x Trainium2 NeuronCores.

Data-parallel over batch: B=32 -> 4 batches per core. Per core the recurrent
state h [4, C=256, K=128] lives in SBUF as hT [K=128 partitions, (b,c)=1024
free] in bf16.  All matmuls are bf16 (1 HW pass + 1 cyc/row vs fp32's 2
passes at 4 cyc/row, and ~80ns LDWEIGHTS instead of ~440ns).

Bulk elementwise work is on DVE: gamma*h uses tensor_tensor (2x bf16 DVE
mode, ~250ns per [128,256]); h_new and the h_tilde accumulation need
scalar_tensor_tensor (per-batch scalar / accum_out), which only runs 1x.
h_tilde is computed from m = gamma*h (not h_new) using the identity
ht = (q_e.q_next)*LG + sum_c q_next*m with the dot products precomputed on
the host, so the next step's gate chain never waits for h_new.
The gate-critical small ops are kept OFF the congested DVE queue so they
never wait behind bulk work: the Z23/U4 constant adds ride the PSUM
accumulation groups as I128 pass-through matmuls on PE, the u psum->SBUF
move is an ACT copy (same queue as the gamma sigmoids that consume it),
and the tiny LG product runs on the otherwise-idle GpSimd.  (Bulk work on
GpSimd is net negative -- it shares SBUF ports with DVE.)

q rows are broadcast across all 128 partitions by DMA (stride-0 source via
AP.partition_broadcast), one 16-step window (4MB) at a time, double
buffered -- the descriptors fan out over all 16 DMA engines, so no compute
engine spends cycles on replication.

The 4 batches are processed as TWO independent 2-batch streams per step so
stream B's gate chain (PE matmul -> sigmoid -> LG -> W4b matmul -> u)
overlaps stream A's state-update tail.  Emission order is deliberately
per-stream sequential: the Tile scheduler follows it closely, and
cross-stream reorderings (hoisting gate matmuls, interleaving tails)
measurably regress.  h_tilde columns accumulate into a persistent HT_all
buffer [K, 4*(T+1)]; gate matmuls read their [K,2] slice directly and the
y head (W5 + sigmoid + reduce) runs once, batched, after the loop.
"""

import numpy as np

B, S = 32, 128
NUM_Q, NUM_C = 10000, 256
K = 128
C = NUM_C
NCORES = 8
BL = B // NCORES  # 4 batches per core
T = S - 1  # 127 recurrence steps
QW = 16  # q broadcast window, steps
NWIN = S // QW

_cache = {}


def _build():
    import concourse.bass as bass  # noqa: F401
    import concourse.mybir as mybir
    import concourse.tile as tile
    from concourse import bacc

    fp32 = mybir.dt.float32
    bf16 = mybir.dt.bfloat16
    AF = mybir.ActivationFunctionType
    OP = mybir.AluOpType

    nc = bacc.Bacc()

    # ---------------- DRAM I/O ----------------
    d = {}

    def din(name, shape, dt_=bf16):
        t = nc.dram_tensor(name, shape, dt_, kind="ExternalInput")
        d[name] = t
        return t

    din("eT", [K, S * BL])       # e_emb^T, free layout (s, b) s-major
    din("atT", [K, S * BL])
    din("itT", [K, S * BL])
    din("a_row", [1, S * BL])
    din("h0T4", [K, BL * C])
    q_dram = nc.dram_tensor("qD", [S, BL * C], bf16, kind="ExternalInput")
    for w in ["W1a", "W1b", "W2a2", "W2b2", "W2c2", "W2d2",
              "W3a", "W3b", "W3c", "W3d", "W4a", "W4b", "W4c",
              "W5a", "W5b"]:
        din(w, [K, K])
    for w in ["w1c", "b1r", "b2r2", "b3r", "b4r", "b5r"]:
        din(w, [1, K])
    din("ones512", [1, 512])
    din("dD", [1, T * BL])
    din("I128", [K, K])
    din("ones128c", [K, 1])
    y_dram = nc.dram_tensor("y_out", [1, BL * T], fp32, kind="ExternalOutput")

    from contextlib import ExitStack

    with tile.TileContext(nc) as tc, ExitStack() as ctx:
        singles = ctx.enter_context(tc.tile_pool(name="singles", bufs=1))
        state = ctx.enter_context(tc.tile_pool(name="state", bufs=1))
        sm = ctx.enter_context(tc.tile_pool(name="sm", bufs=3))
        qw = ctx.enter_context(tc.tile_pool(name="qw", bufs=2))
        pp = ctx.enter_context(tc.tile_pool(name="pp", bufs=2, space="PSUM"))
        psm = ctx.enter_context(tc.tile_pool(name="psm", bufs=2, space="PSUM"))

        # ---------------- load everything to SBUF ----------------
        sb = {}
        for name, dt_ in d.items():
            if name == "h0T4":
                continue  # loaded straight into the state tile below
            t_ = singles.tile(list(dt_.shape), dt_.dtype, tag=name)
            nc.sync.dma_start(out=t_[:], in_=dt_[:])
            sb[name] = t_

        # recurrent state h, DMA'd straight from the prepped h0 tile
        s_h = state.tile([K, BL * C], bf16, tag="h")
        nc.sync.dma_start(out=s_h[:], in_=d["h0T4"][:])

        # q windows: all 128 partitions get a copy of q rows [16w, 16w+16)
        qwin = [None] * NWIN

        def qwin_load(w):
            wt = qw.tile([K, QW * BL * C], bf16, tag="qwin")
            src = q_dram[w * QW:(w + 1) * QW, :].partition_broadcast(K)
            nc.sync.dma_start(out=wt[:], in_=src)
            qwin[w] = wt

        qwin_load(0)
        qwin_load(1)

        # d[t,b] = q_t . q_{t+1} broadcast to all partitions (tiny, one DMA)
        s_d = state.tile([K, T * BL], bf16, tag="dT")
        nc.sync.dma_start(out=s_d[:],
                          in_=d["dD"][:].partition_broadcast(K))

        def qsl(t):
            # [K, 1024] replicated q row for step t
            base = (t % QW) * BL * C
            return qwin[t // QW][:, base:base + BL * C]

        # collapse the ~30 input-DMA dependencies
        tc.strict_bb_all_engine_barrier()

        s_gam = state.tile([K, BL * C], bf16, tag="gam")
        s_m = state.tile([K, BL * C], bf16, tag="m")
        # h_tilde history: block t (cols 4t:4t+4) = h_tilde at step t
        s_HT = state.tile([K, (T + 1) * BL], bf16, tag="HT")

        # ---------------- precompute: allT, Z23, U4, Y5 ----------------
        p_all = pp.tile([K, 512], fp32, tag="pbig", bufs=1)
        nc.tensor.matmul(out=p_all[:], lhsT=sb["W1a"][:], rhs=sb["eT"][:],
                         start=True, stop=False)
        nc.tensor.matmul(out=p_all[:], lhsT=sb["W1b"][:], rhs=sb["atT"][:],
                         start=False, stop=False)
        nc.tensor.matmul(out=p_all[:], lhsT=sb["w1c"][:], rhs=sb["a_row"][:],
                         start=False, stop=False)
        nc.tensor.matmul(out=p_all[:], lhsT=sb["b1r"][:],
                         rhs=sb["ones512"][:], start=False, stop=True)
        s_allT = singles.tile([K, 512], bf16, tag="allT")
        nc.vector.tensor_copy(out=s_allT[:], in_=p_all[:])

        # Z23[k, t, g, b2, b]: gate g in {2,3}, stream b2, batch-in-stream b
        s_Z23 = singles.tile([K, T, 2, 2, 2], bf16, tag="Z23")

        def precompute_z(Wpre, Wit, Wlearn, brow, g):
            ptile = pp.tile([K, T * BL], fp32, tag="pbig", bufs=1)
            nc.tensor.matmul(out=ptile[:], lhsT=sb[Wit][:],
                             rhs=sb["itT"][:, 0:T * BL], start=True, stop=False)
            nc.tensor.matmul(out=ptile[:, BL:T * BL], lhsT=sb[Wpre][:],
                             rhs=s_allT[:, 0:(T - 1) * BL],
                             start=False, stop=False, skip_group_check=True)
            nc.tensor.matmul(out=ptile[:], lhsT=sb[Wlearn][:],
                             rhs=s_allT[:, 0:T * BL], start=False, stop=False)
            nc.tensor.matmul(out=ptile[:], lhsT=sb[brow][:],
                             rhs=sb["ones512"][:, 0:T * BL], start=False,
                             stop=True)
            nc.vector.tensor_copy(
                out=s_Z23[:, :, g, :, :],
                in_=ptile[:].rearrange("k (t b2 b) -> k t b2 b", b2=2, b=2))

        precompute_z("W2a2", "W2b2", "W2c2", "b2r2", 0)
        precompute_z("W3a", "W3b", "W3c", "b3r", 1)

        # U4[k, (t,b)] = it@W4c + b4
        p_u4 = pp.tile([K, T * BL], fp32, tag="pbig", bufs=1)
        nc.tensor.matmul(out=p_u4[:], lhsT=sb["W4c"][:],
                         rhs=sb["itT"][:, 0:T * BL], start=True, stop=False)
        nc.tensor.matmul(out=p_u4[:], lhsT=sb["b4r"][:],
                         rhs=sb["ones512"][:, 0:T * BL], start=False, stop=True)
        s_U4 = singles.tile([K, T * BL], bf16, tag="U4")
        nc.vector.tensor_copy(out=s_U4[:], in_=p_u4[:])

        # Y5[k, (t,b)] = e_emb[t+1]@W5a + b5
        p_y5 = pp.tile([K, T * BL], fp32, tag="pbig", bufs=1)
        nc.tensor.matmul(out=p_y5[:], lhsT=sb["W5a"][:],
                         rhs=sb["eT"][:, BL:S * BL], start=True, stop=False)
        nc.tensor.matmul(out=p_y5[:], lhsT=sb["b5r"][:],
                         rhs=sb["ones512"][:, 0:T * BL], start=False, stop=True)
        s_Y5 = singles.tile([K, T * BL], fp32, tag="Y5")
        nc.vector.tensor_copy(out=s_Y5[:], in_=p_y5[:])

        # ---------------- h_tilde init (with q_0) ----------------
        for b in range(BL):
            cs = slice(b * C, (b + 1) * C)
            nc.vector.scalar_tensor_tensor(
                out=s_m[:, cs], in0=s_h[:, cs], scalar=0.0,
                in1=qsl(0)[:, cs], op0=OP.bypass, op1=OP.mult,
                accum_out=s_HT[:, b:b + 1])

        # ---------------- the recurrence (two 2-batch streams) ----------
        for t in range(T):
            if t % QW == 0 and t > 0 and (t // QW + 1) < NWIN:
                qwin_load(t // QW + 1)

            ps = psm.tile([K, 16], fp32, tag="small")
            deferred_hn = []
            for s2 in range(2):
                o = s2 * 8
                ht_sl = s_HT[:, t * BL + 2 * s2:t * BL + 2 * s2 + 2]
                nc.tensor.matmul(out=ps[:, o:o + 2], lhsT=sb["W2d2"][:],
                                 rhs=ht_sl, start=True, stop=False)
                nc.tensor.matmul(out=ps[:, o + 2:o + 4], lhsT=sb["W3d"][:],
                                 rhs=ht_sl, start=True, stop=False)
                # += Z23[t] on PE (identity pass-through), so no DVE add sits
                # on the gate-critical path
                nc.tensor.matmul(out=ps[:, o:o + 4], lhsT=sb["I128"][:],
                                 rhs=s_Z23[:, t, :, s2, :], start=False,
                                 stop=True, skip_group_check=True)
                # gamma_f preact for this stream's two batches
                pPs = pp.tile([K, 512], fp32, tag=f"pP{s2}")
                nc.tensor.matmul(out=pPs[:], lhsT=sb["W4a"][:],
                                 rhs=s_h[:, s2 * 512:(s2 + 1) * 512],
                                 start=True, stop=True)

                s23 = sm.tile([K, 4], bf16, tag=f"s23{s2}")
                nc.scalar.activation(out=s23[:], in_=ps[:, o:o + 4],
                                     func=AF.Sigmoid)
                # LG on the otherwise-idle GpSimd queue: never waits behind
                # DVE bulk work
                LGT = sm.tile([K, 2], bf16, tag=f"LGT{s2}")
                nc.gpsimd.tensor_mul(out=LGT[:], in0=s23[:, 0:2],
                                     in1=s23[:, 2:4])
                t1 = sm.tile([K, 2], bf16, tag=f"t1{s2}")
                nc.gpsimd.tensor_mul(
                    out=t1[:], in0=LGT[:],
                    in1=s_d[:, t * BL + 2 * s2:t * BL + 2 * s2 + 2])

                # u = LG @ W4b + U4[t] (U4 added on PE); psum -> SBUF move on
                # ACT, same queue as the gamma sigmoids that consume it
                nc.tensor.matmul(out=ps[:, o + 4:o + 6], lhsT=sb["W4b"][:],
                                 rhs=LGT[:], start=True, stop=False)
                nc.tensor.matmul(
                    out=ps[:, o + 4:o + 6], lhsT=sb["I128"][:],
                    rhs=s_U4[:, t * BL + 2 * s2:t * BL + 2 * s2 + 2],
                    start=False, stop=True)
                uT = sm.tile([K, 2], fp32, tag=f"uT{s2}")
                nc.scalar.copy(out=uT[:], in_=ps[:, o + 4:o + 6])

                tmp = sm.tile([K, 2], fp32, tag=f"htmp{s2}")
                for b in range(2):
                    gb = 2 * s2 + b
                    cs = slice(gb * C, (gb + 1) * C)
                    nc.scalar.activation(out=s_gam[:, cs],
                                         in_=pPs[:, b * C:(b + 1) * C],
                                         func=AF.Sigmoid, bias=uT[:, b:b + 1])
                    # m = gamma * h
                    nc.vector.tensor_mul(out=s_m[:, cs], in0=s_gam[:, cs],
                                         in1=s_h[:, cs])
                    # sum_c q_{t+1}*m (h_tilde minus the d*LG correction,
                    # so it does not wait for h_new)
                    nc.vector.scalar_tensor_tensor(
                        out=s_gam[:, cs], in0=s_m[:, cs], scalar=0.0,
                        in1=qsl(t + 1)[:, cs], op0=OP.bypass, op1=OP.mult,
                        accum_out=tmp[:, b:b + 1])
                # h_tilde = d*LG + sum_c q_{t+1}*m  (back-to-back on DVE)
                col0 = (t + 1) * BL + 2 * s2
                nc.vector.tensor_add(out=s_HT[:, col0:col0 + 2],
                                     in0=tmp[:], in1=t1[:])
                for b in range(2):
                    gb = 2 * s2 + b
                    deferred_hn.append((slice(gb * C, (gb + 1) * C), LGT, b))

            # h_new = q_e * LG + m for all batches, after both streams'
            # critical tails: only the next step's W4a@h needs these
            for cs, LGT, b in deferred_hn:
                nc.vector.scalar_tensor_tensor(
                    out=s_h[:, cs], in0=qsl(t)[:, cs],
                    scalar=LGT[:, b:b + 1], in1=s_m[:, cs],
                    op0=OP.mult, op1=OP.add)

        # ---------------- y head, batched over all steps ----------------
        p_y = pp.tile([K, T * BL], fp32, tag="pbig", bufs=1)
        nc.tensor.matmul(out=p_y[:], lhsT=sb["W5b"][:],
                         rhs=s_HT[:, BL:(T + 1) * BL], start=True, stop=True)
        tY = singles.tile([K, T * BL], fp32, tag="tY")
        nc.vector.tensor_add(out=tY[:], in0=p_y[:], in1=s_Y5[:])
        sY = singles.tile([K, T * BL], bf16, tag="sY")
        nc.scalar.activation(out=sY[:], in_=tY[:], func=AF.Sigmoid)
        p_ys = psm.tile([1, T * BL], fp32, tag="yacc", bufs=1)
        nc.tensor.matmul(out=p_ys[:], lhsT=sb["ones128c"][:], rhs=sY[:],
                         start=True, stop=True)
        s_y = singles.tile([1, T * BL], fp32, tag="yout")
        nc.vector.tensor_copy(out=s_y[:], in_=p_ys[:])
        nc.sync.dma_start(out=y_dram[:], in_=s_y[:])

    nc.compile()
    return nc


def _prep_inputs(inputs):
    """Host-side sharding + layout prep. Returns per-core input dicts."""
    import ml_dtypes

    bf = ml_dtypes.bfloat16
    f32 = np.float32
    e_idx = np.asarray(inputs["e_data"]).astype(np.int64)
    at_idx = np.asarray(inputs["at_data"]).astype(np.int64)
    it_idx = np.asarray(inputs["it_data"]).astype(np.int64)
    a_data = np.asarray(inputs["a_data"], dtype=f32)
    q_matrix = np.asarray(inputs["q_matrix"], dtype=f32)
    e_E = np.asarray(inputs["e_E"], dtype=bf)
    at_E = np.asarray(inputs["at_E"], dtype=bf)
    it_E = np.asarray(inputs["it_E"], dtype=bf)
    W1 = np.asarray(inputs["W1"], dtype=f32)
    W2 = np.asarray(inputs["W2"], dtype=f32)
    W3 = np.asarray(inputs["W3"], dtype=f32)
    W4 = np.asarray(inputs["W4"], dtype=f32)
    W5 = np.asarray(inputs["W5"], dtype=f32)
    h0 = np.asarray(inputs["h0"], dtype=f32)

    def bfc(x):
        return np.ascontiguousarray(np.asarray(x, dtype=bf))

    shared = {
        "W1a": bfc(W1[0:K]), "W1b": bfc(W1[K:2 * K]),
        "w1c": bfc(W1[2 * K:].sum(0)[None, :]),
        "b1r": bfc(np.asarray(inputs["b1"], dtype=f32)[None, :]),
        "W2a2": bfc(2 * W2[0:K]), "W2b2": bfc(2 * W2[K:2 * K]),
        "W2c2": bfc(2 * W2[2 * K:3 * K]), "W2d2": bfc(2 * W2[3 * K:]),
        "b2r2": bfc(2 * np.asarray(inputs["b2"], dtype=f32)[None, :]),
        "W3a": bfc(W3[0:K]), "W3b": bfc(W3[K:2 * K]),
        "W3c": bfc(W3[2 * K:3 * K]), "W3d": bfc(W3[3 * K:]),
        "b3r": bfc(np.asarray(inputs["b3"], dtype=f32)[None, :]),
        "W4a": bfc(W4[0:K]), "W4b": bfc(W4[K:2 * K]), "W4c": bfc(W4[2 * K:]),
        "b4r": bfc(np.asarray(inputs["b4"], dtype=f32)[None, :]),
        "W5a": bfc(W5[0:K]), "W5b": bfc(W5[K:]),
        "b5r": bfc(np.asarray(inputs["b5"], dtype=f32)[None, :]),
        "ones512": bfc(np.ones((1, 512), f32)),
        "ones128c": bfc(np.ones((K, 1), f32)),
        "I128": bfc(np.eye(K, dtype=f32)),
        "h0T4": bfc(np.tile(np.ascontiguousarray(h0.T), (1, BL))),
    }

    in_maps = []
    for g in range(NCORES):
        bg = slice(g * BL, (g + 1) * BL)
        e_emb = e_E[e_idx[bg]]          # [4, S, K] bf16
        at_emb = at_E[at_idx[bg]]
        it_emb = it_E[it_idx[bg]]
        q_all = q_matrix[e_idx[bg]]     # [4, S, C] f32
        m = dict(shared)
        # [K, (s, b)] s-major layouts
        m["eT"] = bfc(e_emb.transpose(2, 1, 0).reshape(K, S * BL))
        m["atT"] = bfc(at_emb.transpose(2, 1, 0).reshape(K, S * BL))
        m["itT"] = bfc(it_emb.transpose(2, 1, 0).reshape(K, S * BL))
        m["qD"] = bfc(q_all.transpose(1, 0, 2).reshape(S, BL * C))
        dmat = (q_all[:, :-1] * q_all[:, 1:]).sum(-1)  # [BL, T]
        m["dD"] = bfc(dmat.T.reshape(1, T * BL))
        m["a_row"] = bfc(a_data[bg].T.reshape(1, S * BL))
        in_maps.append(m)
    return in_maps


def _run(inputs, trace=False):
    from concourse.bass_utils import run_bass_kernel_spmd

    if "nc" not in _cache:
        _cache["nc"] = _build()
    nc = _cache["nc"]
    in_maps = _prep_inputs(inputs)
    res = run_bass_kernel_spmd(nc, in_maps, core_ids=list(range(NCORES)),
                               trace=trace)
    pred = np.zeros((B, S), np.float32)
    for g in range(NCORES):
        y = res.results[g]["y_out"].reshape(T, BL)  # [t, b]
        pred[g * BL:(g + 1) * BL, 1:] = y.T / K
    return pred, res


def kernel(**inputs):
    return _run(inputs)[0]
